# revision 1
# baseline (speedup 1.0000x reference)
"""Self-contained Trainium2 Bass kernel for nn_AutoRegressive_88837103551116.

2-layer LSTM (HID=64) over ragged sequences: warmup pass over x (per-sample
lengths), then autoregressive decode over [dense(h_top_final), context_t].
Pure data-parallel over 8 NeuronCores (batch 512 -> 64 per core).

Device algorithm (per core):
  - slot s computes layer0 @ step s and layer1 @ step s-1 (layer stagger) so
    both layers' gates share each tick's instructions
  - states [feature, batch]: rb [128,B]=[h0;h1], cc [64,2B]=[c0|c1]
  - gates via 8 small matmuls/tick into 2 PSUM banks (one accumulation group
    open per bank at a time; recurrence-independent matmuls lead each tick so
    the PE runs ahead); biases and the ragged-sequence c-freeze (+/-BIG added
    to i/f pre-activations past each sample's length) are folded into extra
    stationary-weight rows
  - h at the last valid step is captured into hkeep via copy_predicated with
    uint8 mask streams (off the recurrence critical path)
  - host side: input transposes/stream building, output -999 masking
"""
import sys

import numpy as np

try:
    import concourse.bass as bass
except ImportError:
    sys.path.insert(0, "/opt/trn_rl_repo")
    import concourse.bass as bass

import contextlib
import json

import concourse.tile as tile
from concourse import mybir
from concourse.bass_utils import run_bass_kernel_spmd

N_CORES = 8
TW = 512
TC = 512



H = 64
IN = 16
F = 8
C = 8
BIG = 50.0


def build_weights(Wih0, Whh0, bih0, bhh0, Wih1, Whh1, bih1, bhh1, Wd, bd):
    """Build all stationary lhsT matrices (shared across cores). fp32."""
    b0 = bih0 + bhh0
    b1 = bih1 + bhh1
    maskcol_if = np.concatenate([np.full(H, -BIG), np.full(H, BIG)]).astype(np.float32)

    def stack_l0(gate_rows, xw, bias, mask):
        # lhsT [18, 128]: rows 0:16 = xw^T, 16 = mask, 17 = bias
        out = np.zeros((18, 128), np.float32)
        out[0:xw.shape[1], :] = xw[gate_rows].T
        out[16] = mask
        out[17] = bias[gate_rows]
        return out

    gi = slice(0, 128)   # i,f rows
    gg = slice(128, 256)  # g,o rows
    W = {}
    W["w0x_if"] = stack_l0(gi, Wih0, b0, maskcol_if)
    W["w0x_go"] = stack_l0(gg, Wih0, b0, np.zeros(128, np.float32))
    W["w0h_if"] = Whh0[gi].T.copy()   # [64,128]
    W["w0h_go"] = Whh0[gg].T.copy()
    W["w1_if"] = np.concatenate([Wih1[gi].T, Whh1[gi].T], 0)  # [128,128]
    W["w1_go"] = np.concatenate([Wih1[gg].T, Whh1[gg].T], 0)
    W["wc_if"] = np.stack([b1[gi], maskcol_if]).astype(np.float32)  # [2,128]
    W["wc_go"] = b1[gg].reshape(1, 128).astype(np.float32)
    # decode l0: input rows 0:8 = ctx weights (cols 8:16 of Wih0), elem separate
    def stack_l0_dec(gate_rows, mask):
        out = np.zeros((18, 128), np.float32)
        out[0:8, :] = Wih0[gate_rows, 8:16].T
        out[16] = mask
        out[17] = b0[gate_rows]
        return out
    W["d0x_if"] = stack_l0_dec(gi, maskcol_if)
    W["d0x_go"] = stack_l0_dec(gg, np.zeros(128, np.float32))
    W["d0e_if"] = Wih0[gi, 0:8].T.copy()  # [8,128]
    W["d0e_go"] = Wih0[gg, 0:8].T.copy()
    W["wdT"] = Wd.T.copy()  # [64,8]
    W["bd"] = bd.reshape(8, 1).copy()
    for k in W:
        W[k] = np.ascontiguousarray(W[k], np.float32)
    return W


def build_streams(x, lengths_x, context, lengths_ctx):
    """Per-core streams. x [B,TW,16], context [B,TC,8]. B arbitrary."""
    B = x.shape[0]
    TW = x.shape[1]
    TC = context.shape[1]
    SW, SD = 528, 512
    CW, CD = SW // 8, SD // 8

    s_idx = np.arange(SW)
    mw = (s_idx[:, None] < lengths_x[None, :]).astype(np.float32)  # [SW,B]
    mw1 = np.zeros_like(mw)
    mw1[1:] = mw[:-1]

    WA = np.zeros((CW, 18, 8, B), np.float32)
    xt = np.transpose(x, (1, 2, 0))  # [TW,16,B]
    WA[:, 0:16].reshape(CW * 8 * 16, B)
    WA_r = WA.reshape(CW, 18, 8, B)
    for c in range(CW):
        for t in range(8):
            s = c * 8 + t
            if s < TW:
                WA_r[c, 0:16, t, :] = xt[s]
            WA_r[c, 16, t, :] = 1.0 - mw[s]
            WA_r[c, 17, t, :] = 1.0
    WC = np.zeros((CW, 2, 8, B), np.float32)
    WC[:, 0] = 1.0
    WC[:, 1] = (1.0 - mw1).reshape(CW, 8, B)
    NMw = np.zeros((CW, 128, 8, B), np.uint8)
    NMw[:, 0:64] = mw.reshape(CW, 8, 1, B).transpose(0, 2, 1, 3)
    NMw[:, 64:128] = mw1.reshape(CW, 8, 1, B).transpose(0, 2, 1, 3)

    md0 = np.zeros((SD, B), np.float32)
    md0[0:TC - 1] = 1.0     # l0 steps 0..510 active; 511 pad frozen
    md1 = np.ones((SD, B), np.float32)
    md1[0] = 0.0            # freeze l1 at slot 0
    DA = np.zeros((CD, 18, 8, B), np.float32)
    ctxt = np.transpose(context, (1, 2, 0))  # [TC,8,B]
    DA_r = DA
    for c in range(CD):
        for t in range(8):
            s = c * 8 + t
            if s < TC - 1:
                DA_r[c, 0:8, t, :] = ctxt[s]
            DA_r[c, 16, t, :] = 1.0 - md0[s]
            DA_r[c, 17, t, :] = 1.0
    DC = np.zeros((CD, 2, 8, B), np.float32)
    DC[:, 0] = 1.0
    DC[:, 1] = (1.0 - md1).reshape(CD, 8, B)
    NMd = np.zeros((CD, 128, 8, B), np.uint8)
    NMd[:, 0:64] = (1.0 - md0).reshape(CD, 8, 1, B).transpose(0, 2, 1, 3)
    NMd[:, 64:128] = (1.0 - md1).reshape(CD, 8, 1, B).transpose(0, 2, 1, 3)

    def pad1(a):
        return np.concatenate([a, np.zeros_like(a[:1])], 0)
    return dict(
        wa=pad1(WA.reshape(CW, 18, 8 * B)),
        wcs=pad1(WC.reshape(CW, 2, 8 * B)),
        nmw=pad1(NMw.reshape(CW, 128, 8 * B)),
        da=pad1(DA.reshape(CD, 18, 8 * B)),
        dcs=pad1(DC.reshape(CD, 2, 8 * B)),
        nmd=pad1(NMd.reshape(CD, 128, 8 * B)),
    )


def post_outputs(YE, YD, bd, lengths_ctx, TC):
    """YE [8,B], YD [CD,8,8,B] -> out [B,TC,8] with -999 padding."""
    B = YE.shape[1]
    out = np.zeros((B, TC, F), np.float32)
    out[:, 0, :] = YE.T
    ysd = YD.transpose(0, 2, 1, 3).reshape(512, F, B)  # [slot, F, B]
    # ys_t = slot t+1 for t = 0..510
    out[:, 1:TC, :] = ysd[1:TC].transpose(2, 0, 1) + bd[None, None, :]
    valid = np.arange(TC)[None, :] < lengths_ctx[:, None]
    return np.where(valid[:, :, None], out, np.float32(-999.0))



import contextlib

import concourse.bass as bass
import concourse.tile as tile
from concourse import mybir

F32 = mybir.dt.float32
U8 = mybir.dt.uint8
AF = mybir.ActivationFunctionType

B = 64
H = 64
SW = 528   # warmup slots (padded; needs >= 513)
SD = 512   # decode slots (l0 steps 0..510 + 1 pad)
CW = SW // 8
CD = SD // 8


def build_nc(repeat=1, static=False):
    nc = bass.Bass("TRN2", target_bir_lowering=False, debug=False)
    dt = F32

    d = {}
    d["wa"] = nc.dram_tensor("wa", [CW + 1, 18, 512], dt, kind="ExternalInput")
    d["wcs"] = nc.dram_tensor("wcs", [CW + 1, 2, 512], dt, kind="ExternalInput")
    d["nmw"] = nc.dram_tensor("nmw", [CW + 1, 128, 512], U8, kind="ExternalInput")
    d["da"] = nc.dram_tensor("da", [CD + 1, 18, 512], dt, kind="ExternalInput")
    d["dcs"] = nc.dram_tensor("dcs", [CD + 1, 2, 512], dt, kind="ExternalInput")
    for name, shp in [
        ("w0x_if", [18, 128]), ("w0x_go", [18, 128]),
        ("w0h_if", [64, 128]), ("w0h_go", [64, 128]),
        ("w1_if", [128, 128]), ("w1_go", [128, 128]),
        ("wc_if", [2, 128]), ("wc_go", [1, 128]),
        ("d0x_if", [18, 128]), ("d0x_go", [18, 128]),
        ("d0e_if", [8, 128]), ("d0e_go", [8, 128]),
        ("wdT", [64, 8]), ("bd", [8, 1]),
    ]:
        d[name] = nc.dram_tensor(name, shp, dt, kind="ExternalInput")
    ye = nc.dram_tensor("ye", [8, B], dt, kind="ExternalOutput")
    yd = nc.dram_tensor("yd", [CD, 8, 512], dt, kind="ExternalOutput")

    with tile.TileContext(nc) as tc:
        with (
            tc.tile_pool(name="consts", bufs=1) as consts,
            tc.tile_pool(name="state", bufs=1) as state,
            tc.tile_pool(name="stream", bufs=1) as stream,
            tc.tile_pool(name="work", bufs=3) as work,
            tc.tile_pool(name="psum", bufs=2, space="PSUM") as psum,
            tc.tile_pool(name="outp", bufs=1, space="PSUM") as outp,
        ):
            W = {}
            for name in ["w0x_if", "w0x_go", "w0h_if", "w0h_go", "w1_if",
                         "w1_go", "wc_if", "wc_go", "d0x_if", "d0x_go",
                         "d0e_if", "d0e_go", "bd"]:
                t = consts.tile(list(d[name].shape), dt, tag=name, name="w_" + name)
                nc.sync.dma_start(out=t, in_=d[name][:, :])
                W[name] = t
            wdT_t = consts.tile([128, 8], dt, tag="wdT", name="w_wdT")
            nc.sync.dma_start(out=wdT_t[64:128, :], in_=d["wdT"][:, :])
            W["wdT"] = wdT_t

            rb = [state.tile([128, B], dt, tag=f"rb{i}", name=f"rb{i}") for i in range(2)]
            cc = [state.tile([64, 2 * B], dt, tag=f"cc{i}", name=f"cc{i}") for i in range(2)]
            for i in range(2):
                nc.vector.memset(rb[i], 0.0)
                nc.vector.memset(cc[i], 0.0)

            saA = stream.tile([18, 512], dt, tag="saA")
            saB = stream.tile([18, 512], dt, tag="saB")
            scA = stream.tile([2, 512], dt, tag="scA")
            scB = stream.tile([2, 512], dt, tag="scB")
            nmA = stream.tile([128, 512], U8, tag="nmA")
            nmB = stream.tile([128, 512], U8, tag="nmB")
            elem = state.tile([8, B], dt, tag="elem")
            hkeep = state.tile([128, B], dt, tag="hkeep")
            nc.vector.memset(hkeep, 0.0)

            def tick(sl, sa, sc, nm, decode):
                par = sl % 2
                rbp, rbn = rb[par], rb[1 - par]
                ccp, ccn_dst = cc[par], cc[1 - par]
                t8 = sl % 8
                colB = slice(t8 * B, (t8 + 1) * B)
                megaIF = psum.tile([128, 2 * B], dt, tag="megaIF", name="megaIF")
                megaGO = psum.tile([128, 2 * B], dt, tag="megaGO", name="megaGO")

                wx_if = W["d0x_if"] if decode else W["w0x_if"]
                wx_go = W["d0x_go"] if decode else W["w0x_go"]

                # Gate matmuls. Two PSUM banks (IF / GO), at most one open
                # accumulation group per bank; the rbp-independent leading MMs
                # let the PE run ahead during the previous tick's tail.
                if decode:
                    nc.tensor.matmul(megaIF[:, 0:B], W["d0e_if"], elem, start=True, stop=False)
                    nc.tensor.matmul(megaGO[:, 0:B], W["d0e_go"], elem, start=True, stop=False)
                    nc.tensor.matmul(megaIF[:, 0:B], wx_if, sa[0:18, colB], start=False, stop=False)
                    nc.tensor.matmul(megaGO[:, 0:B], wx_go, sa[0:18, colB], start=False, stop=False)
                else:
                    nc.tensor.matmul(megaIF[:, 0:B], wx_if, sa[0:18, colB], start=True, stop=False)
                    nc.tensor.matmul(megaGO[:, 0:B], wx_go, sa[0:18, colB], start=True, stop=False)
                nc.tensor.matmul(megaIF[:, 0:B], W["w0h_if"], rbp[0:64, :], start=False, stop=True)
                nc.tensor.matmul(megaIF[:, B:2 * B], W["wc_if"], sc[0:2, colB], start=True, stop=False)
                nc.tensor.matmul(megaIF[:, B:2 * B], W["w1_if"], rbp[:, :], start=False, stop=True)
                nc.tensor.matmul(megaGO[:, 0:B], W["w0h_go"], rbp[0:64, :], start=False, stop=True)
                nc.tensor.matmul(megaGO[:, B:2 * B], W["wc_go"], sc[0:1, colB], start=True, stop=False)
                nc.tensor.matmul(megaGO[:, B:2 * B], W["w1_go"], rbp[:, :], start=False, stop=True)

                # activations (sif in PSUM: exempts t1/t2 from equal-base rule)
                sif = psum.tile([128, 2 * B], dt, tag="sif", name="sif")
                tg = work.tile([64, 2 * B], dt, tag="tg", name="tg")
                so = work.tile([64, 2 * B], dt, tag="so", name="so")
                nc.scalar.activation(sif, megaIF[:, :], AF.Sigmoid)
                nc.scalar.activation(tg, megaGO[0:64, :], AF.Tanh)
                nc.scalar.activation(so, megaGO[64:128, :], AF.Sigmoid)

                # elementwise
                t1 = work.tile([64, 2 * B], dt, tag="t1", name="t1")
                t2 = work.tile([64, 2 * B], dt, tag="t2", name="t2")
                th = work.tile([64, 2 * B], dt, tag="th", name="th")
                nc.vector.tensor_mul(t1, sif[0:64, :], tg)
                nc.vector.tensor_mul(t2, sif[64:128, :], ccp)
                nc.vector.tensor_add(ccn_dst, t1, t2)
                nc.scalar.activation(th, ccn_dst, AF.Tanh)
                nc.vector.tensor_mul(rbn[0:64, :], so[:, 0:B], th[:, 0:B])
                nc.gpsimd.tensor_mul(rbn[64:128, :], so[:, B:2 * B], th[:, B:2 * B])

                if nm is not None:
                    # capture h at each sample's last active slot (off the
                    # recurrence critical path)
                    nc.vector.copy_predicated(hkeep, nm[:, colB], rbn)
                return rbn

            def loop(n):
                # static: python-unrolled; else: hardware For_i
                if static:
                    return contextlib.nullcontext(enumerate(range(n)))
                return None

            rep_cm = tc.For_i(0, repeat, 1) if repeat > 1 else contextlib.nullcontext()
            with rep_cm:
                # ================= warmup =================
                nc.sync.dma_start(out=saA, in_=d["wa"][0, :, :])
                nc.sync.dma_start(out=scA, in_=d["wcs"][0, :, :])
                nc.sync.dma_start(out=nmA, in_=d["nmw"][0, :, :])
                def warm_body(j, i1, i2, first=False):
                    nc.sync.dma_start(out=saB, in_=d["wa"][i1, :, :])
                    nc.sync.dma_start(out=scB, in_=d["wcs"][i1, :, :])
                    nc.sync.dma_start(out=nmB, in_=d["nmw"][i1, :, :])
                    for sl in range(8):
                        tick(sl, saA, scA, nmA, False)
                        if first and sl == 0:
                            nc.vector.memset(rb[1][64:128, :], 0.0)
                    nc.sync.dma_start(out=saA, in_=d["wa"][i2, :, :])
                    nc.sync.dma_start(out=scA, in_=d["wcs"][i2, :, :])
                    nc.sync.dma_start(out=nmA, in_=d["nmw"][i2, :, :])
                    for sl in range(8, 16):
                        tick(sl, saB, scB, nmB, False)

                if static:
                    for j in range(CW // 2):
                        warm_body(j, j * 2 + 1, j * 2 + 2, first=(j == 0))
                else:
                    warm_body(0, 1, 2, first=True)
                    with tc.For_i(1, CW // 2, 1, hint_engines=(mybir.EngineType.PE,)) as j:
                        warm_body(j, nc.snap(j * 2 + 1), nc.snap(j * 2 + 2))

                # ================= elem =================
                nc.vector.tensor_copy(rb[0], hkeep)
                pe = outp.tile([8, B], dt, tag="ops", name="pe")
                nc.tensor.matmul(pe, W["wdT"][64:128, :], rb[0][64:128, :], start=True, stop=True)
                nc.scalar.activation(elem, pe, AF.Identity, bias=W["bd"][:, 0:1])
                nc.sync.dma_start(out=ye[:, :], in_=elem)

                # ================= decode =================
                nc.sync.dma_start(out=saA, in_=d["da"][0, :, :])
                nc.sync.dma_start(out=scA, in_=d["dcs"][0, :, :])
                def dec_body(j, i0, i1, i2, first=False):
                    nc.sync.dma_start(out=saB, in_=d["da"][i1, :, :])
                    nc.sync.dma_start(out=scB, in_=d["dcs"][i1, :, :])
                    ops = outp.tile([8, 512], dt, tag="ops", name="ops")
                    for sl in range(8):
                        rbn = tick(sl, saA, scA, None, True)
                        if first and sl == 0:
                            nc.vector.tensor_copy(rb[1][64:128, :], rb[0][64:128, :])
                        nc.tensor.matmul(ops[:, sl * B:(sl + 1) * B], W["wdT"][64:128, :],
                                         rbn[64:128, :], start=True, stop=True)
                    oso = work.tile([8, 512], dt, tag="oso", name="oso")
                    nc.scalar.copy(oso, ops)
                    nc.sync.dma_start(out=yd[i0, :, :], in_=oso)
                    nc.sync.dma_start(out=saA, in_=d["da"][i2, :, :])
                    nc.sync.dma_start(out=scA, in_=d["dcs"][i2, :, :])
                    ops2 = outp.tile([8, 512], dt, tag="ops2", name="ops2")
                    for sl in range(8, 16):
                        rbn = tick(sl, saB, scB, None, True)
                        nc.tensor.matmul(ops2[:, (sl - 8) * B:(sl - 7) * B], W["wdT"][64:128, :],
                                         rbn[64:128, :], start=True, stop=True)
                    oso2 = work.tile([8, 512], dt, tag="oso2", name="oso2")
                    nc.scalar.copy(oso2, ops2)
                    nc.sync.dma_start(out=yd[i1, :, :], in_=oso2)

                if static:
                    for j in range(CD // 2):
                        dec_body(j, j * 2, j * 2 + 1, j * 2 + 2, first=(j == 0))
                else:
                    dec_body(0, 0, 1, 2, first=True)
                    with tc.For_i(1, CD // 2, 1, hint_engines=(mybir.EngineType.PE,)) as j:
                        dec_body(j, nc.snap(j * 2), nc.snap(j * 2 + 1), nc.snap(j * 2 + 2))

    return nc


def legalize_waits(nc, max_waits=1):
    """walrus codegen caps semaphore waits per instruction; move extras onto
    NoOp instructions inserted immediately before (same engine)."""
    j = json.loads(mybir.module_to_json_bytes(nc.m))
    for fn in j.get("functions", []):
        for blk in fn.get("blocks", []):
            out = []
            for inst in blk.get("instructions", []):
                si = inst.get("sync_info") or {}
                waits = si.get("on_wait") or []
                if len(waits) > max_waits:
                    keep, extra = waits[-max_waits:], waits[:-max_waits]
                    for k, w in enumerate(extra):
                        out.append({"name": f"{inst['name']}-wsp{k}",
                                    "opcode": "NoOp", "engine": inst["engine"],
                                    "ins": [], "outs": [],
                                    "sync_info": {"on_wait": [w], "on_update": []}})
                    si = dict(si); si["on_wait"] = keep
                    inst = dict(inst); inst["sync_info"] = si
                out.append(inst)
            blk["instructions"] = out
    nc.m = mybir.module_from_json_bytes(json.dumps(j).encode())
    return nc


_NC_CACHE = {}


def _get_nc(repeat=1):
    if repeat not in _NC_CACHE:
        nc = build_nc(repeat)
        legalize_waits(nc)
        _NC_CACHE[repeat] = nc
    return _NC_CACHE[repeat]


def build_in_maps(x, lengths_x, context, lengths_ctx,
                  Wih0, Whh0, bih0, bhh0, Wih1, Whh1, bih1, bhh1, Wd, bd):
    Wt = build_weights(Wih0.astype(np.float32), Whh0.astype(np.float32),
                       bih0.astype(np.float32), bhh0.astype(np.float32),
                       Wih1.astype(np.float32), Whh1.astype(np.float32),
                       bih1.astype(np.float32), bhh1.astype(np.float32),
                       Wd.astype(np.float32), bd.astype(np.float32))
    Bn = x.shape[0] // N_CORES
    in_maps = []
    for core in range(N_CORES):
        sl = slice(core * Bn, (core + 1) * Bn)
        st = build_streams(np.ascontiguousarray(x[sl], dtype=np.float32),
                           np.asarray(lengths_x[sl], dtype=np.int64),
                           np.ascontiguousarray(context[sl], dtype=np.float32),
                           np.asarray(lengths_ctx[sl], dtype=np.int64))
        m = dict(st)
        m.pop("nmd", None)
        m.update(Wt)
        in_maps.append(m)
    return in_maps


def kernel(x, lengths_x, context, lengths_ctx,
           Wih0, Whh0, bih0, bhh0, Wih1, Whh1, bih1, bhh1, Wd, bd):
    x = np.asarray(x)
    context = np.asarray(context)
    lengths_x = np.asarray(lengths_x)
    lengths_ctx = np.asarray(lengths_ctx)
    in_maps = build_in_maps(x, lengths_x, context, lengths_ctx,
                            np.asarray(Wih0), np.asarray(Whh0), np.asarray(bih0),
                            np.asarray(bhh0), np.asarray(Wih1), np.asarray(Whh1),
                            np.asarray(bih1), np.asarray(bhh1), np.asarray(Wd),
                            np.asarray(bd))
    nc = _get_nc(1)
    res = run_bass_kernel_spmd(nc, in_maps, core_ids=list(range(N_CORES)))
    Bn = x.shape[0] // N_CORES
    outs = []
    bd32 = np.asarray(bd, dtype=np.float32)
    for core in range(N_CORES):
        sl = slice(core * Bn, (core + 1) * Bn)
        YE = res.results[core]["ye"]
        YD = res.results[core]["yd"].reshape(CD, 8, 8, Bn)
        outs.append(post_outputs(YE, YD, bd32,
                                 np.asarray(lengths_ctx[sl], dtype=np.int64), TC))
    return np.concatenate(outs, axis=0).astype(np.float32)



# revision 5
# speedup vs baseline: 1.7020x; 1.7020x over previous
"""Self-contained Trainium2 Bass kernel for nn_AutoRegressive_88837103551116.

2-layer LSTM (HID=64) over ragged sequences: warmup pass over x (per-sample
lengths), then autoregressive decode over [dense(h_top_final), context_t].
Pure data-parallel over 8 NeuronCores (batch 512 -> 64 per core).

Per-core design (v2, latency-optimized):
- bf16 matmul operands everywhere (4x PE throughput vs fp32; PSUM accumulates
  fp32).  End-to-end rel err ~8e-5 vs the fp32 reference.
- 4 recurrent matmuls per tick (layer0-h, layer1-h per gate bank).  The
  x-stream and layer1-bias contributions are batched per 4-tick PSUM chunk as
  single strided matmuls; the per-tick matmuls accumulate onto those closed
  regions (start=False stop=True, verified exact on HW).
- no +/-BIG freeze masks: c and h are captured at each sample's last valid
  step with copy_predicated (mask streams), off the recurrence critical path.
- h state lives in an 8-slot ring [128, 8, B]; decode output projections are
  per-tick matmuls into a per-chunk [8, 512] PSUM tile (stable baseline
  pattern), copied out once per chunk.
- critical chain per tick: PE(h-matmuls) -> ACT(sigmoid) -> DVE(t2,t1,c') ->
  ACT(tanh) -> DVE(h'-halves) -> PE.
"""
import sys

import numpy as np

try:
    import concourse.bass as bass
except ImportError:
    sys.path.insert(0, "/opt/trn_rl_repo")
    import concourse.bass as bass

import contextlib
import json

import ml_dtypes

import concourse.tile as tile
from concourse import mybir
from concourse.bass_utils import run_bass_kernel_spmd

BF16_NP = ml_dtypes.bfloat16
F32 = mybir.dt.float32
BF16 = mybir.dt.bfloat16
U8 = mybir.dt.uint8
AF = mybir.ActivationFunctionType

N_CORES = 8
B = 64
H = 64
TW = 512
TC = 512
SW = 528
SD = 512
CW = SW // 8   # 66
CD = SD // 8   # 64


def build_weights(Wih0, Whh0, bih0, bhh0, Wih1, Whh1, bih1, bhh1, Wd, bd):
    b0 = bih0 + bhh0
    b1 = bih1 + bhh1
    gi = slice(0, 128)
    gg = slice(128, 256)

    def bf(a):
        return np.ascontiguousarray(a, dtype=np.float32).astype(BF16_NP)

    W = {}
    for tag, gr in (("if", gi), ("go", gg)):
        wx = np.zeros((17, 128), np.float32)
        wx[0] = b0[gr]
        wx[1:17] = Wih0[gr].T
        W[f"wx_{tag}"] = bf(wx)
        dx = np.zeros((9, 128), np.float32)
        dx[0] = b0[gr]
        dx[1:9] = Wih0[gr, 8:16].T
        W[f"dx_{tag}"] = bf(dx)
        W[f"de_{tag}"] = bf(Wih0[gr, 0:8].T)
        W[f"w0h_{tag}"] = bf(Whh0[gr].T)
        W[f"w1_{tag}"] = bf(np.concatenate([Wih1[gr].T, Whh1[gr].T], 0))
        W[f"b1_{tag}"] = bf(b1[gr].reshape(1, 128))
    W["wdT"] = bf(Wd.T)
    W["bd"] = np.ascontiguousarray(bd.reshape(8, 1), np.float32)
    return W


def build_streams(x, lengths_x, context, lengths_ctx):
    Bn = x.shape[0]
    s_idx = np.arange(SW)
    mw = (s_idx[:, None] < lengths_x[None, :]).astype(np.uint8)   # [SW,B]
    mw1 = np.zeros_like(mw)
    mw1[1:] = mw[:-1]

    # warmup stream [CW,17,8*B]: row 0 ones (bias), rows 1:17 x
    WA = np.zeros((CW, 17, 8, Bn), np.float32)
    xt = np.transpose(x, (1, 2, 0))  # [TW,16,B]
    for s in range(SW):
        c, t = divmod(s, 8)
        if s < TW:
            WA[c, 1:17, t] = xt[s]
        WA[c, 0, t] = 1.0
    # h-capture mask [CW,128,8*B]: rows 0:64 mw (h0), rows 64:128 mw1 (h1)
    NM = np.zeros((CW, 128, 8, Bn), np.uint8)
    NM[:, 0:64] = mw.reshape(CW, 8, 1, Bn).transpose(0, 2, 1, 3)
    NM[:, 64:128] = mw1.reshape(CW, 8, 1, Bn).transpose(0, 2, 1, 3)
    # c-capture mask [CW,64,8*2B]: per tick cols 0:B mw (c0), B:2B mw1 (c1)
    NC = np.zeros((CW, 64, 8, 2, Bn), np.uint8)
    NC[:, :, :, 0, :] = mw.reshape(CW, 8, 1, Bn).transpose(0, 2, 1, 3)
    NC[:, :, :, 1, :] = mw1.reshape(CW, 8, 1, Bn).transpose(0, 2, 1, 3)

    # decode stream [CD,9,8*B]: row 0 ones, rows 1:9 ctx
    DA = np.zeros((CD, 9, 8, Bn), np.float32)
    ctxt = np.transpose(context, (1, 2, 0))  # [TC,8,B]
    for s in range(SD):
        c, t = divmod(s, 8)
        if s < TC - 1:
            DA[c, 1:9, t] = ctxt[s]
        DA[c, 0, t] = 1.0

    def pad1(a):
        return np.concatenate([a, np.zeros_like(a[:1])], 0)

    return dict(
        wa=pad1(WA.reshape(CW, 17, 8 * Bn).astype(BF16_NP)),
        nmw=pad1(NM.reshape(CW, 128, 8 * Bn)),
        ncm=pad1(NC.reshape(CW, 64, 16 * Bn)),
        da=pad1(DA.reshape(CD, 9, 8 * Bn).astype(BF16_NP)),
    )


def post_outputs(YE, YD, bd, lengths_ctx):
    """YE [8,B]; YD [CD,8,8,B] flat over ticks: flat[t] = Wd @ h1_dec(t-1)."""
    Bn = YE.shape[1]
    ys = np.zeros((SD, 8, Bn), np.float32)  # ys[t] = Wd @ h1_dec(t)
    flat = YD.reshape(CD * 8, 8, Bn)
    ys[0:SD - 1] = flat[1:SD]
    out = np.zeros((Bn, TC, 8), np.float32)
    out[:, 0, :] = YE.T
    out[:, 1:TC, :] = ys[0:TC - 1].transpose(2, 0, 1) + bd[None, None, :]
    valid = np.arange(TC)[None, :] < lengths_ctx[:, None]
    return np.where(valid[:, :, None], out, np.float32(-999.0))


def build_nc(repeat=1):
    nc = bass.Bass("TRN2", target_bir_lowering=False, debug=False)

    d = {}
    d["wa"] = nc.dram_tensor("wa", [CW + 1, 17, 512], BF16, kind="ExternalInput")
    d["nmw"] = nc.dram_tensor("nmw", [CW + 1, 128, 512], U8, kind="ExternalInput")
    d["ncm"] = nc.dram_tensor("ncm", [CW + 1, 64, 1024], U8, kind="ExternalInput")
    d["da"] = nc.dram_tensor("da", [CD + 1, 9, 512], BF16, kind="ExternalInput")
    for name, shp in [
        ("wx_if", [17, 128]), ("wx_go", [17, 128]),
        ("dx_if", [9, 128]), ("dx_go", [9, 128]),
        ("de_if", [8, 128]), ("de_go", [8, 128]),
        ("w0h_if", [64, 128]), ("w0h_go", [64, 128]),
        ("w1_if", [128, 128]), ("w1_go", [128, 128]),
        ("b1_if", [1, 128]), ("b1_go", [1, 128]),
        ("wdT", [64, 8]),
    ]:
        d[name] = nc.dram_tensor(name, shp, BF16, kind="ExternalInput")
    d["bd"] = nc.dram_tensor("bd", [8, 1], F32, kind="ExternalInput")
    ye = nc.dram_tensor("ye", [8, B], F32, kind="ExternalOutput")
    yd = nc.dram_tensor("yd", [CD, 8, 512], F32, kind="ExternalOutput")

    with tile.TileContext(nc) as tc:
        with (
            tc.tile_pool(name="consts", bufs=1) as consts,
            tc.tile_pool(name="state", bufs=1) as state,
            tc.tile_pool(name="stream", bufs=1) as stream,
            tc.tile_pool(name="work", bufs=3) as work,
            tc.tile_pool(name="gpsum", bufs=2, space="PSUM") as gpsum,
            tc.tile_pool(name="outp", bufs=2, space="PSUM") as outp,
            tc.tile_pool(name="spsum", bufs=2, space="PSUM") as spsum,
        ):
            W = {}
            for name in ["wx_if", "wx_go", "dx_if", "dx_go", "de_if", "de_go",
                         "w0h_if", "w0h_go", "w1_if", "w1_go", "b1_if", "b1_go",
                         "bd"]:
                t = consts.tile(list(d[name].shape),
                                F32 if name == "bd" else BF16,
                                tag=name, name="w_" + name)
                nc.sync.dma_start(out=t, in_=d[name][:, :])
                W[name] = t
            wdT_t = consts.tile([128, 8], BF16, tag="wdT", name="w_wdT")
            nc.sync.dma_start(out=wdT_t[64:128, :], in_=d["wdT"][:, :])
            W["wdT"] = wdT_t

            ring = state.tile([128, 8, B], BF16, tag="ring", name="ring")
            cc = [state.tile([64, 2 * B], F32, tag=f"cc{i}", name=f"cc{i}") for i in range(2)]
            hkeep = state.tile([128, B], BF16, tag="hkeep", name="hkeep")
            ckeep = state.tile([64, 2 * B], F32, tag="ckeep", name="ckeep")
            elem = state.tile([8, B], F32, tag="elem", name="elem")

            saA = stream.tile([17, 512], BF16, tag="saA", name="saA")
            saB = stream.tile([17, 512], BF16, tag="saB", name="saB")
            nmA = stream.tile([128, 512], U8, tag="nmA", name="nmA")
            nmB = stream.tile([128, 512], U8, tag="nmB", name="nmB")
            ncA = stream.tile([64, 1024], U8, tag="ncA", name="ncA")
            ncB = stream.tile([64, 1024], U8, tag="ncB", name="ncB")
            daA = stream.tile([9, 512], BF16, tag="daA", name="daA")
            daB = stream.tile([9, 512], BF16, tag="daB", name="daB")
            elem256 = state.tile([8, 256], BF16, tag="elem256", name="elem256")

            def quad_mms(bIF, bGO, sa, decode, q):
                """per-4-tick-chunk batched matmuls: x-stream + l1 bias (+elem)."""
                cols = slice(q * 4 * B, (q + 1) * 4 * B)
                if decode:
                    nc.tensor.matmul(bIF[:, :, 0:B], W["dx_if"], sa[0:9, cols],
                                     start=True, stop=True)
                    nc.tensor.matmul(bGO[:, :, 0:B], W["dx_go"], sa[0:9, cols],
                                     start=True, stop=True)
                    nc.tensor.matmul(bIF[:, :, 0:B], W["de_if"], elem256[:, 0:256],
                                     start=False, stop=True, skip_group_check=True)
                    nc.tensor.matmul(bGO[:, :, 0:B], W["de_go"], elem256[:, 0:256],
                                     start=False, stop=True, skip_group_check=True)
                else:
                    nc.tensor.matmul(bIF[:, :, 0:B], W["wx_if"], sa[0:17, cols],
                                     start=True, stop=True)
                    nc.tensor.matmul(bGO[:, :, 0:B], W["wx_go"], sa[0:17, cols],
                                     start=True, stop=True)
                nc.tensor.matmul(bIF[:, :, B:2 * B], W["b1_if"],
                                 sa[0:1, cols], start=True, stop=True)
                nc.tensor.matmul(bGO[:, :, B:2 * B], W["b1_go"],
                                 sa[0:1, cols], start=True, stop=True)

            def tick(s, bIF, bGO, k, nm, ncmt, decode, ops=None):
                """one LSTM tick; k = index within 4-tick psum chunk."""
                t8 = s % 8
                colB = slice(t8 * B, (t8 + 1) * B)
                col2B = slice(t8 * 2 * B, (t8 + 1) * 2 * B)
                slot = t8
                nslot = (t8 + 1) % 8
                ccp, ccn = cc[s % 2], cc[(s + 1) % 2]

                # recurrent matmuls accumulate onto the batched x/bias results
                nc.tensor.matmul(bIF[:, k, 0:B], W["w0h_if"], ring[0:64, slot, :],
                                 start=False, stop=True, skip_group_check=True)
                nc.tensor.matmul(bIF[:, k, B:2 * B], W["w1_if"], ring[:, slot, :],
                                 start=False, stop=True, skip_group_check=True)
                nc.tensor.matmul(bGO[:, k, 0:B], W["w0h_go"], ring[0:64, slot, :],
                                 start=False, stop=True, skip_group_check=True)
                nc.tensor.matmul(bGO[:, k, B:2 * B], W["w1_go"], ring[:, slot, :],
                                 start=False, stop=True, skip_group_check=True)

                sif = spsum.tile([128, 2 * B], F32, tag="sif", name="sif")
                tg = work.tile([64, 2 * B], F32, tag="tg", name="tg")
                so = work.tile([64, 2 * B], F32, tag="so", name="so")
                t1 = work.tile([64, 2 * B], F32, tag="t1", name="t1")
                t2 = work.tile([64, 2 * B], F32, tag="t2", name="t2")
                th = work.tile([64, 2 * B], F32, tag="th", name="th")

                nc.scalar.activation(sif, bIF[:, k, :], AF.Sigmoid)
                nc.scalar.activation(tg, bGO[0:64, k, :], AF.Tanh)
                nc.scalar.activation(so, bGO[64:128, k, :], AF.Sigmoid)
                nc.vector.tensor_mul(t2, sif[64:128, :], ccp)
                nc.vector.tensor_mul(t1, sif[0:64, :], tg)
                nc.vector.tensor_add(ccn, t1, t2)
                nc.scalar.activation(th, ccn, AF.Tanh)
                nc.vector.tensor_mul(ring[0:64, nslot, :], so[:, 0:B], th[:, 0:B])
                nc.vector.tensor_mul(ring[64:128, nslot, :], so[:, B:2 * B], th[:, B:2 * B])

                if ops is not None:
                    nc.tensor.matmul(ops[:, t8 * B:(t8 + 1) * B], W["wdT"][64:128, :],
                                     ring[64:128, nslot, :], start=True, stop=True)
                if nm is not None:
                    # capture h/c at each sample's last valid step
                    nc.vector.copy_predicated(hkeep, nm[:, colB], ring[:, nslot, :])
                    nc.vector.copy_predicated(ckeep, ncmt[:, col2B], ccn)

            def chunk8(cbase, sa, nm, ncmt, decode, first=False):
                ops = None
                if decode:
                    ops = outp.tile([8, 512], F32, tag="ops", name="ops")
                for q in range(2):
                    bIF = gpsum.tile([128, 4, 2 * B], F32, tag="bIF", name="bIF")
                    bGO = gpsum.tile([128, 4, 2 * B], F32, tag="bGO", name="bGO")
                    quad_mms(bIF, bGO, sa, decode, q)
                    for k in range(4):
                        s = cbase + q * 4 + k
                        tick(s, bIF, bGO, k, nm, ncmt, decode, ops=ops)
                        if first and q == 0 and k == 0:
                            if decode:
                                nc.vector.tensor_copy(ring[64:128, 1, :], hkeep[64:128, :])
                                nc.vector.tensor_copy(cc[1][:, B:2 * B], ckeep[:, B:2 * B])
                            else:
                                nc.vector.memset(ring[64:128, 1, :], 0.0)
                                nc.vector.memset(cc[1][:, B:2 * B], 0.0)
                if decode:
                    oso = work.tile([8, 512], F32, tag="oso", name="oso")
                    nc.scalar.copy(oso, ops)
                    return oso
                return None

            rep_cm = tc.For_i(0, repeat, 1) if repeat > 1 else contextlib.nullcontext()
            with rep_cm:
                # init state
                nc.vector.memset(ring, 0.0)
                nc.vector.memset(cc[0], 0.0)
                nc.vector.memset(cc[1], 0.0)
                nc.vector.memset(hkeep, 0.0)
                nc.vector.memset(ckeep, 0.0)

                # ============ warmup ============
                nc.sync.dma_start(out=saA, in_=d["wa"][0, :, :])
                nc.sync.dma_start(out=nmA, in_=d["nmw"][0, :, :])
                nc.sync.dma_start(out=ncA, in_=d["ncm"][0, :, :])

                def warm_body(j, i1, i2, first=False):
                    nc.sync.dma_start(out=saB, in_=d["wa"][i1, :, :])
                    nc.sync.dma_start(out=nmB, in_=d["nmw"][i1, :, :])
                    nc.sync.dma_start(out=ncB, in_=d["ncm"][i1, :, :])
                    chunk8(0, saA, nmA, ncA, False, first=first)
                    nc.sync.dma_start(out=saA, in_=d["wa"][i2, :, :])
                    nc.sync.dma_start(out=nmA, in_=d["nmw"][i2, :, :])
                    nc.sync.dma_start(out=ncA, in_=d["ncm"][i2, :, :])
                    chunk8(8, saB, nmB, ncB, False)

                warm_body(0, 1, 2, first=True)
                with tc.For_i(1, CW // 2, 1, hint_engines=(mybir.EngineType.PE,)) as j:
                    warm_body(j, nc.snap(j * 2 + 1), nc.snap(j * 2 + 2))

                # ============ transition ============
                pe = outp.tile([8, B], F32, tag="ops", name="pe")
                nc.tensor.matmul(pe, W["wdT"][64:128, :], hkeep[64:128, :], start=True, stop=True)
                nc.scalar.activation(elem, pe, AF.Identity, bias=W["bd"][:, 0:1])
                nc.sync.dma_start(out=ye[:, :], in_=elem)
                nc.vector.tensor_copy(ring[:, 0, :], hkeep)
                nc.vector.tensor_copy(cc[0], ckeep)
                for u in range(4):
                    nc.vector.tensor_copy(elem256[:, u * 64:(u + 1) * 64], elem)

                # ============ decode ============
                nc.sync.dma_start(out=daA, in_=d["da"][0, :, :])

                def dec_body(j, i0, i1, i2, first=False):
                    nc.sync.dma_start(out=daB, in_=d["da"][i1, :, :])
                    oso = chunk8(0, daA, None, None, True, first=first)
                    nc.sync.dma_start(out=yd[i0, :, :], in_=oso)
                    nc.sync.dma_start(out=daA, in_=d["da"][i2, :, :])
                    oso2 = chunk8(8, daB, None, None, True)
                    nc.sync.dma_start(out=yd[i1, :, :], in_=oso2)

                dec_body(0, 0, 1, 2, first=True)
                with tc.For_i(1, CD // 2, 1, hint_engines=(mybir.EngineType.PE,)) as j:
                    dec_body(j, nc.snap(j * 2), nc.snap(j * 2 + 1), nc.snap(j * 2 + 2))

    return nc


def legalize_waits(nc, max_waits=1):
    """walrus codegen caps semaphore waits per instruction; move extras onto
    NoOp instructions inserted immediately before (same engine)."""
    j = json.loads(mybir.module_to_json_bytes(nc.m))
    for fn in j.get("functions", []):
        for blk in fn.get("blocks", []):
            out = []
            for inst in blk.get("instructions", []):
                si = inst.get("sync_info") or {}
                waits = si.get("on_wait") or []
                if len(waits) > max_waits:
                    keep, extra = waits[-max_waits:], waits[:-max_waits]
                    for k, w in enumerate(extra):
                        out.append({"name": f"{inst['name']}-wsp{k}",
                                    "opcode": "NoOp", "engine": inst["engine"],
                                    "ins": [], "outs": [],
                                    "sync_info": {"on_wait": [w], "on_update": []}})
                    si = dict(si); si["on_wait"] = keep
                    inst = dict(inst); inst["sync_info"] = si
                out.append(inst)
            blk["instructions"] = out
    nc.m = mybir.module_from_json_bytes(json.dumps(j).encode())
    return nc


_NC_CACHE = {}


def _get_nc(repeat=1):
    if repeat not in _NC_CACHE:
        nc = build_nc(repeat)
        legalize_waits(nc)
        _NC_CACHE[repeat] = nc
    return _NC_CACHE[repeat]


def build_in_maps(x, lengths_x, context, lengths_ctx,
                  Wih0, Whh0, bih0, bhh0, Wih1, Whh1, bih1, bhh1, Wd, bd):
    Wt = build_weights(*[np.asarray(a, np.float32) for a in
                         (Wih0, Whh0, bih0, bhh0, Wih1, Whh1, bih1, bhh1, Wd, bd)])
    Bn = x.shape[0] // N_CORES
    in_maps = []
    for core in range(N_CORES):
        sl = slice(core * Bn, (core + 1) * Bn)
        st = build_streams(np.ascontiguousarray(x[sl], np.float32),
                           np.asarray(lengths_x[sl], np.int64),
                           np.ascontiguousarray(context[sl], np.float32),
                           np.asarray(lengths_ctx[sl], np.int64))
        m = dict(st)
        m.update(Wt)
        in_maps.append(m)
    return in_maps


def kernel(x, lengths_x, context, lengths_ctx,
           Wih0, Whh0, bih0, bhh0, Wih1, Whh1, bih1, bhh1, Wd, bd):
    x = np.asarray(x)
    context = np.asarray(context)
    lengths_x = np.asarray(lengths_x)
    lengths_ctx = np.asarray(lengths_ctx)
    in_maps = build_in_maps(x, lengths_x, context, lengths_ctx,
                            np.asarray(Wih0), np.asarray(Whh0), np.asarray(bih0),
                            np.asarray(bhh0), np.asarray(Wih1), np.asarray(Whh1),
                            np.asarray(bih1), np.asarray(bhh1), np.asarray(Wd),
                            np.asarray(bd))
    nc = _get_nc(1)
    res = run_bass_kernel_spmd(nc, in_maps, core_ids=list(range(N_CORES)))
    Bn = x.shape[0] // N_CORES
    outs = []
    bd32 = np.asarray(bd, dtype=np.float32)
    for core in range(N_CORES):
        sl = slice(core * Bn, (core + 1) * Bn)
        YE = res.results[core]["ye"]
        YD = res.results[core]["yd"].reshape(CD, 8, 8, Bn)
        outs.append(post_outputs(YE, YD, bd32,
                                 np.asarray(lengths_ctx[sl], dtype=np.int64)))
    return np.concatenate(outs, axis=0).astype(np.float32)


# revision 8
# speedup vs baseline: 2.1562x; 1.2669x over previous
"""Self-contained Trainium2 Bass kernel for nn_AutoRegressive_88837103551116.

2-layer LSTM (HID=64) over ragged sequences: warmup pass over x (per-sample
lengths), then autoregressive decode over [dense(h_top_final), context_t].
Pure data-parallel over 8 NeuronCores (batch 512 -> 64 per core).

Per-core design (v2, latency-optimized):
- bf16 matmul operands everywhere (4x PE throughput vs fp32; PSUM accumulates
  fp32).  End-to-end rel err ~8e-5 vs the fp32 reference.
- 4 recurrent matmuls per tick (layer0-h, layer1-h per gate bank).  The
  x-stream and layer1-bias contributions are batched per 4-tick PSUM chunk as
  single strided matmuls; the per-tick matmuls accumulate onto those closed
  regions (start=False stop=True, verified exact on HW).
- no +/-BIG freeze masks: c and h are captured at each sample's last valid
  step with copy_predicated (mask streams), off the recurrence critical path.
- h state lives in an 8-slot ring [128, 8, B]; decode output projections are
  per-tick matmuls into a per-chunk [8, 512] PSUM tile (stable baseline
  pattern), copied out once per chunk.
- critical chain per tick: PE(h-matmuls) -> ACT(sigmoid) -> DVE(t2,t1,c') ->
  ACT(tanh) -> DVE(h'-halves) -> PE.
"""
import sys

import numpy as np

try:
    import concourse.bass as bass
except ImportError:
    sys.path.insert(0, "/opt/trn_rl_repo")
    import concourse.bass as bass

import contextlib
import json

import ml_dtypes

import concourse.tile as tile
from concourse import mybir
from concourse.bass_utils import run_bass_kernel_spmd

BF16_NP = ml_dtypes.bfloat16
F32 = mybir.dt.float32
BF16 = mybir.dt.bfloat16
U8 = mybir.dt.uint8
AF = mybir.ActivationFunctionType

N_CORES = 8
B = 64
H = 64
TW = 512
TC = 512
SW = 528
SD = 512
CW = SW // 8   # 66
CD = SD // 8   # 64


def build_weights(Wih0, Whh0, bih0, bhh0, Wih1, Whh1, bih1, bhh1, Wd, bd):
    b0 = bih0 + bhh0
    b1 = bih1 + bhh1
    gi = slice(0, 128)
    gg = slice(128, 256)

    def bf(a):
        return np.ascontiguousarray(a, dtype=np.float32).astype(BF16_NP)

    W = {}
    for tag, gr in (("if", gi), ("go", gg)):
        wx = np.zeros((17, 128), np.float32)
        wx[0] = b0[gr]
        wx[1:17] = Wih0[gr].T
        W[f"wx_{tag}"] = bf(wx)
        dx = np.zeros((9, 128), np.float32)
        dx[0] = b0[gr]
        dx[1:9] = Wih0[gr, 8:16].T
        W[f"dx_{tag}"] = bf(dx)
        W[f"de_{tag}"] = bf(Wih0[gr, 0:8].T)
        W[f"w0h_{tag}"] = bf(Whh0[gr].T)
        W[f"w1_{tag}"] = bf(np.concatenate([Wih1[gr].T, Whh1[gr].T], 0))
        W[f"b1_{tag}"] = bf(b1[gr].reshape(1, 128))
    W["wdT"] = bf(Wd.T)
    W["bd"] = np.ascontiguousarray(bd.reshape(8, 1), np.float32)
    return W


def build_streams(x, lengths_x, context, lengths_ctx):
    Bn = x.shape[0]
    s_idx = np.arange(SW)
    mw = (s_idx[:, None] < lengths_x[None, :]).astype(np.uint8)   # [SW,B]
    mw1 = np.zeros_like(mw)
    mw1[1:] = mw[:-1]

    # warmup stream [CW,17,8*B]: row 0 ones (bias), rows 1:17 x
    WA = np.zeros((CW, 17, 8, Bn), np.float32)
    xt = np.transpose(x, (1, 2, 0))  # [TW,16,B]
    for s in range(SW):
        c, t = divmod(s, 8)
        if s < TW:
            WA[c, 1:17, t] = xt[s]
        WA[c, 0, t] = 1.0
    # h-capture mask [CW,128,8*B]: rows 0:64 mw (h0), rows 64:128 mw1 (h1)
    NM = np.zeros((CW, 128, 8, Bn), np.uint8)
    NM[:, 0:64] = mw.reshape(CW, 8, 1, Bn).transpose(0, 2, 1, 3)
    NM[:, 64:128] = mw1.reshape(CW, 8, 1, Bn).transpose(0, 2, 1, 3)
    # c-capture mask [CW,64,8*2B]: per tick cols 0:B mw (c0), B:2B mw1 (c1)
    NC = np.zeros((CW, 64, 8, 2, Bn), np.uint8)
    NC[:, :, :, 0, :] = mw.reshape(CW, 8, 1, Bn).transpose(0, 2, 1, 3)
    NC[:, :, :, 1, :] = mw1.reshape(CW, 8, 1, Bn).transpose(0, 2, 1, 3)

    # decode stream [CD,9,8*B]: row 0 ones, rows 1:9 ctx
    DA = np.zeros((CD, 9, 8, Bn), np.float32)
    ctxt = np.transpose(context, (1, 2, 0))  # [TC,8,B]
    for s in range(SD):
        c, t = divmod(s, 8)
        if s < TC - 1:
            DA[c, 1:9, t] = ctxt[s]
        DA[c, 0, t] = 1.0

    def pad1(a):
        return np.concatenate([a, np.zeros_like(a[:1])], 0)

    return dict(
        wa=pad1(WA.reshape(CW, 17, 8 * Bn).astype(BF16_NP)),
        nmw=pad1(NM.reshape(CW, 128, 8 * Bn)),
        ncm=pad1(NC.reshape(CW, 64, 16 * Bn)),
        da=pad1(DA.reshape(CD, 9, 8 * Bn).astype(BF16_NP)),
    )


def post_outputs(YE, YD, bd, lengths_ctx):
    """YE [8,B]; YD [CD,8,8,B] flat over ticks: flat[t] = Wd @ h1_dec(t-1)."""
    Bn = YE.shape[1]
    ys = np.zeros((SD, 8, Bn), np.float32)  # ys[t] = Wd @ h1_dec(t)
    flat = YD.reshape(CD * 8, 8, Bn)
    ys[0:SD - 1] = flat[1:SD]
    out = np.zeros((Bn, TC, 8), np.float32)
    out[:, 0, :] = YE.T
    out[:, 1:TC, :] = ys[0:TC - 1].transpose(2, 0, 1) + bd[None, None, :]
    valid = np.arange(TC)[None, :] < lengths_ctx[:, None]
    return np.where(valid[:, :, None], out, np.float32(-999.0))


def build_nc(repeat=1):
    nc = bass.Bass("TRN2", target_bir_lowering=False, debug=False)

    d = {}
    d["wa"] = nc.dram_tensor("wa", [CW + 1, 17, 512], BF16, kind="ExternalInput")
    d["nmw"] = nc.dram_tensor("nmw", [CW + 1, 128, 512], U8, kind="ExternalInput")
    d["ncm"] = nc.dram_tensor("ncm", [CW + 1, 64, 1024], U8, kind="ExternalInput")
    d["da"] = nc.dram_tensor("da", [CD + 1, 9, 512], BF16, kind="ExternalInput")
    for name, shp in [
        ("wx_if", [17, 128]), ("wx_go", [17, 128]),
        ("dx_if", [9, 128]), ("dx_go", [9, 128]),
        ("de_if", [8, 128]), ("de_go", [8, 128]),
        ("w0h_if", [64, 128]), ("w0h_go", [64, 128]),
        ("w1_if", [128, 128]), ("w1_go", [128, 128]),
        ("b1_if", [1, 128]), ("b1_go", [1, 128]),
        ("wdT", [64, 8]),
    ]:
        d[name] = nc.dram_tensor(name, shp, BF16, kind="ExternalInput")
    d["bd"] = nc.dram_tensor("bd", [8, 1], F32, kind="ExternalInput")
    ye = nc.dram_tensor("ye", [8, B], F32, kind="ExternalOutput")
    yd = nc.dram_tensor("yd", [CD, 8, 512], F32, kind="ExternalOutput")

    with tile.TileContext(nc) as tc:
        with (
            tc.tile_pool(name="consts", bufs=1) as consts,
            tc.tile_pool(name="state", bufs=1) as state,
            tc.tile_pool(name="stream", bufs=1) as stream,
            tc.tile_pool(name="work", bufs=4) as work,
            tc.tile_pool(name="gpsum", bufs=2, space="PSUM") as gpsum,
            tc.tile_pool(name="outp", bufs=2, space="PSUM") as outp,
            tc.tile_pool(name="spsum", bufs=2, space="PSUM") as spsum,
        ):
            W = {}
            for name in ["wx_if", "wx_go", "dx_if", "dx_go", "de_if", "de_go",
                         "w0h_if", "w0h_go", "w1_if", "w1_go", "b1_if", "b1_go",
                         "bd"]:
                t = consts.tile(list(d[name].shape),
                                F32 if name == "bd" else BF16,
                                tag=name, name="w_" + name)
                nc.sync.dma_start(out=t, in_=d[name][:, :])
                W[name] = t
            wdT_t = consts.tile([128, 8], BF16, tag="wdT", name="w_wdT")
            nc.sync.dma_start(out=wdT_t[64:128, :], in_=d["wdT"][:, :])
            W["wdT"] = wdT_t

            ring = state.tile([128, 8, B], BF16, tag="ring", name="ring")
            cc = [state.tile([64, 2 * B], F32, tag=f"cc{i}", name=f"cc{i}") for i in range(2)]
            hkeep = state.tile([128, B], BF16, tag="hkeep", name="hkeep")
            ckeep = state.tile([64, 2 * B], F32, tag="ckeep", name="ckeep")
            elem = state.tile([8, B], F32, tag="elem", name="elem")

            saA = stream.tile([17, 512], BF16, tag="saA", name="saA")
            saB = stream.tile([17, 512], BF16, tag="saB", name="saB")
            nmA = stream.tile([128, 512], U8, tag="nmA", name="nmA")
            nmB = stream.tile([128, 512], U8, tag="nmB", name="nmB")
            ncA = stream.tile([64, 1024], U8, tag="ncA", name="ncA")
            ncB = stream.tile([64, 1024], U8, tag="ncB", name="ncB")
            daA = stream.tile([9, 512], BF16, tag="daA", name="daA")
            daB = stream.tile([9, 512], BF16, tag="daB", name="daB")
            elem256 = state.tile([8, 256], BF16, tag="elem256", name="elem256")

            def quad_mms(bIF, bGO, sa, decode, q):
                """per-4-tick-chunk batched matmuls: x-stream + l1 bias (+elem)."""
                cols = slice(q * 4 * B, (q + 1) * 4 * B)
                if decode:
                    nc.tensor.matmul(bIF[:, :, 0:B], W["dx_if"], sa[0:9, cols],
                                     start=True, stop=True)
                    nc.tensor.matmul(bGO[:, :, 0:B], W["dx_go"], sa[0:9, cols],
                                     start=True, stop=True)
                    nc.tensor.matmul(bIF[:, :, 0:B], W["de_if"], elem256[:, 0:256],
                                     start=False, stop=True, skip_group_check=True)
                    nc.tensor.matmul(bGO[:, :, 0:B], W["de_go"], elem256[:, 0:256],
                                     start=False, stop=True, skip_group_check=True)
                else:
                    nc.tensor.matmul(bIF[:, :, 0:B], W["wx_if"], sa[0:17, cols],
                                     start=True, stop=True)
                    nc.tensor.matmul(bGO[:, :, 0:B], W["wx_go"], sa[0:17, cols],
                                     start=True, stop=True)
                nc.tensor.matmul(bIF[:, :, B:2 * B], W["b1_if"],
                                 sa[0:1, cols], start=True, stop=True)
                nc.tensor.matmul(bGO[:, :, B:2 * B], W["b1_go"],
                                 sa[0:1, cols], start=True, stop=True)

            def tick(s, bIF, bGO, k, nm, ncmt, decode, ops=None):
                """one LSTM tick; k = index within 4-tick psum chunk."""
                t8 = s % 8
                colB = slice(t8 * B, (t8 + 1) * B)
                col2B = slice(t8 * 2 * B, (t8 + 1) * 2 * B)
                slot = t8
                nslot = (t8 + 1) % 8
                ccp, ccn = cc[s % 2], cc[(s + 1) % 2]

                # recurrent matmuls accumulate onto the batched x/bias results
                nc.tensor.matmul(bIF[:, k, 0:B], W["w0h_if"], ring[0:64, slot, :],
                                 start=False, stop=True, skip_group_check=True)
                nc.tensor.matmul(bIF[:, k, B:2 * B], W["w1_if"], ring[:, slot, :],
                                 start=False, stop=True, skip_group_check=True)
                nc.tensor.matmul(bGO[:, k, 0:B], W["w0h_go"], ring[0:64, slot, :],
                                 start=False, stop=True, skip_group_check=True)
                nc.tensor.matmul(bGO[:, k, B:2 * B], W["w1_go"], ring[:, slot, :],
                                 start=False, stop=True, skip_group_check=True)

                sif = spsum.tile([128, 2 * B], F32, tag="sif", name="sif")
                tg = work.tile([64, 2 * B], F32, tag="tg", name="tg")
                so = work.tile([64, 2 * B], F32, tag="so", name="so")
                t1 = work.tile([64, 2 * B], F32, tag="t1", name="t1")
                t2 = work.tile([64, 2 * B], F32, tag="t2", name="t2")
                th = work.tile([64, 2 * B], F32, tag="th", name="th")

                nc.scalar.activation(sif, bIF[:, k, :], AF.Sigmoid)
                nc.scalar.activation(tg, bGO[0:64, k, :], AF.Tanh)
                nc.scalar.activation(so, bGO[64:128, k, :], AF.Sigmoid)
                nc.vector.tensor_mul(t2, sif[64:128, :], ccp)
                nc.vector.tensor_mul(t1, sif[0:64, :], tg)
                nc.vector.tensor_add(ccn, t1, t2)
                nc.scalar.activation(th, ccn, AF.Tanh)
                nc.vector.tensor_mul(ring[0:64, nslot, :], so[:, 0:B], th[:, 0:B])
                nc.vector.tensor_mul(ring[64:128, nslot, :], so[:, B:2 * B], th[:, B:2 * B])

                if ops is not None:
                    nc.tensor.matmul(ops[:, t8 * B:(t8 + 1) * B], W["wdT"][64:128, :],
                                     ring[64:128, nslot, :], start=True, stop=True)
                if nm is not None:
                    # capture h/c at each sample's last valid step
                    nc.vector.copy_predicated(hkeep, nm[:, colB], ring[:, nslot, :])
                    nc.vector.copy_predicated(ckeep, ncmt[:, col2B], ccn)

            def chunk8(cbase, sa, nm, ncmt, decode, first=False):
                ops = None
                if decode:
                    ops = outp.tile([8, 512], F32, tag="ops", name="ops")
                for q in range(2):
                    bIF = gpsum.tile([128, 4, 2 * B], F32, tag="bIF", name="bIF")
                    bGO = gpsum.tile([128, 4, 2 * B], F32, tag="bGO", name="bGO")
                    quad_mms(bIF, bGO, sa, decode, q)
                    for k in range(4):
                        s = cbase + q * 4 + k
                        tick(s, bIF, bGO, k, nm, ncmt, decode, ops=ops)
                        if first and q == 0 and k == 0:
                            if decode:
                                nc.vector.tensor_copy(ring[64:128, 1, :], hkeep[64:128, :])
                                nc.vector.tensor_copy(cc[1][:, B:2 * B], ckeep[:, B:2 * B])
                            else:
                                nc.vector.memset(ring[64:128, 1, :], 0.0)
                                nc.vector.memset(cc[1][:, B:2 * B], 0.0)
                if decode:
                    oso = work.tile([8, 512], F32, tag="oso", name="oso")
                    nc.scalar.copy(oso, ops)
                    return oso
                return None

            rep_cm = tc.For_i(0, repeat, 1) if repeat > 1 else contextlib.nullcontext()
            with rep_cm:
                # init state
                nc.vector.memset(ring, 0.0)
                nc.vector.memset(cc[0], 0.0)
                nc.vector.memset(cc[1], 0.0)
                nc.vector.memset(hkeep, 0.0)
                nc.vector.memset(ckeep, 0.0)

                # ============ warmup ============
                nc.sync.dma_start(out=saA, in_=d["wa"][0, :, :])
                nc.sync.dma_start(out=nmA, in_=d["nmw"][0, :, :])
                nc.sync.dma_start(out=ncA, in_=d["ncm"][0, :, :])

                def warm_body(j, i1, i2, first=False):
                    nc.sync.dma_start(out=saB, in_=d["wa"][i1, :, :])
                    nc.sync.dma_start(out=nmB, in_=d["nmw"][i1, :, :])
                    nc.sync.dma_start(out=ncB, in_=d["ncm"][i1, :, :])
                    chunk8(0, saA, nmA, ncA, False, first=first)
                    nc.sync.dma_start(out=saA, in_=d["wa"][i2, :, :])
                    nc.sync.dma_start(out=nmA, in_=d["nmw"][i2, :, :])
                    nc.sync.dma_start(out=ncA, in_=d["ncm"][i2, :, :])
                    chunk8(8, saB, nmB, ncB, False)

                warm_body(0, 1, 2, first=True)
                with tc.For_i(1, CW // 2, 1, hint_engines=(mybir.EngineType.PE,)) as j:
                    warm_body(j, nc.snap(j * 2 + 1), nc.snap(j * 2 + 2))

                # ============ transition ============
                pe = outp.tile([8, B], F32, tag="ops", name="pe")
                nc.tensor.matmul(pe, W["wdT"][64:128, :], hkeep[64:128, :], start=True, stop=True)
                nc.scalar.activation(elem, pe, AF.Identity, bias=W["bd"][:, 0:1])
                nc.sync.dma_start(out=ye[:, :], in_=elem)
                nc.vector.tensor_copy(ring[:, 0, :], hkeep)
                nc.vector.tensor_copy(cc[0], ckeep)
                for u in range(4):
                    nc.vector.tensor_copy(elem256[:, u * 64:(u + 1) * 64], elem)

                # ============ decode ============
                nc.sync.dma_start(out=daA, in_=d["da"][0, :, :])

                def dec_body(j, i0, i1, i2, first=False):
                    nc.sync.dma_start(out=daB, in_=d["da"][i1, :, :])
                    oso = chunk8(0, daA, None, None, True, first=first)
                    nc.sync.dma_start(out=yd[i0, :, :], in_=oso)
                    nc.sync.dma_start(out=daA, in_=d["da"][i2, :, :])
                    oso2 = chunk8(8, daB, None, None, True)
                    nc.sync.dma_start(out=yd[i1, :, :], in_=oso2)

                dec_body(0, 0, 1, 2, first=True)
                with tc.For_i(1, CD // 2, 1, hint_engines=(mybir.EngineType.PE,)) as j:
                    dec_body(j, nc.snap(j * 2), nc.snap(j * 2 + 1), nc.snap(j * 2 + 2))

    return nc


def legalize_waits(nc, max_waits=1):
    """walrus codegen caps semaphore waits per instruction; move extras onto
    NoOp instructions inserted immediately before (same engine)."""
    j = json.loads(mybir.module_to_json_bytes(nc.m))
    for fn in j.get("functions", []):
        for blk in fn.get("blocks", []):
            out = []
            for inst in blk.get("instructions", []):
                si = inst.get("sync_info") or {}
                waits = si.get("on_wait") or []
                if len(waits) > max_waits:
                    keep, extra = waits[-max_waits:], waits[:-max_waits]
                    for k, w in enumerate(extra):
                        out.append({"name": f"{inst['name']}-wsp{k}",
                                    "opcode": "NoOp", "engine": inst["engine"],
                                    "ins": [], "outs": [],
                                    "sync_info": {"on_wait": [w], "on_update": []}})
                    si = dict(si); si["on_wait"] = keep
                    inst = dict(inst); inst["sync_info"] = si
                out.append(inst)
            blk["instructions"] = out
    nc.m = mybir.module_from_json_bytes(json.dumps(j).encode())
    return nc


_NC_CACHE = {}


def _get_nc(repeat=1):
    if repeat not in _NC_CACHE:
        nc = build_nc(repeat)
        legalize_waits(nc)
        _NC_CACHE[repeat] = nc
    return _NC_CACHE[repeat]


def build_in_maps(x, lengths_x, context, lengths_ctx,
                  Wih0, Whh0, bih0, bhh0, Wih1, Whh1, bih1, bhh1, Wd, bd):
    Wt = build_weights(*[np.asarray(a, np.float32) for a in
                         (Wih0, Whh0, bih0, bhh0, Wih1, Whh1, bih1, bhh1, Wd, bd)])
    Bn = x.shape[0] // N_CORES
    in_maps = []
    for core in range(N_CORES):
        sl = slice(core * Bn, (core + 1) * Bn)
        st = build_streams(np.ascontiguousarray(x[sl], np.float32),
                           np.asarray(lengths_x[sl], np.int64),
                           np.ascontiguousarray(context[sl], np.float32),
                           np.asarray(lengths_ctx[sl], np.int64))
        m = dict(st)
        m.update(Wt)
        in_maps.append(m)
    return in_maps


def kernel(x, lengths_x, context, lengths_ctx,
           Wih0, Whh0, bih0, bhh0, Wih1, Whh1, bih1, bhh1, Wd, bd):
    x = np.asarray(x)
    context = np.asarray(context)
    lengths_x = np.asarray(lengths_x)
    lengths_ctx = np.asarray(lengths_ctx)
    in_maps = build_in_maps(x, lengths_x, context, lengths_ctx,
                            np.asarray(Wih0), np.asarray(Whh0), np.asarray(bih0),
                            np.asarray(bhh0), np.asarray(Wih1), np.asarray(Whh1),
                            np.asarray(bih1), np.asarray(bhh1), np.asarray(Wd),
                            np.asarray(bd))
    nc = _get_nc(1)
    res = run_bass_kernel_spmd(nc, in_maps, core_ids=list(range(N_CORES)))
    Bn = x.shape[0] // N_CORES
    outs = []
    bd32 = np.asarray(bd, dtype=np.float32)
    for core in range(N_CORES):
        sl = slice(core * Bn, (core + 1) * Bn)
        YE = res.results[core]["ye"]
        YD = res.results[core]["yd"].reshape(CD, 8, 8, Bn)
        outs.append(post_outputs(YE, YD, bd32,
                                 np.asarray(lengths_ctx[sl], dtype=np.int64)))
    return np.concatenate(outs, axis=0).astype(np.float32)
